# revision 56
# baseline (speedup 1.0000x reference)
"""Trainium2 Bass/Tile kernel for nn_MultiHeadAttention (B=4, S=2048, D=1024,
H=16, Dh=64, fp32), SPMD across 8 NeuronCores.

Sharding: core c -> batch c//2, head-half c%2 (8 heads per core).
Host pre-transposes each batch slice to [D, S] and casts to bf16.

v2.9: software-pipelined half-stages.  The kernel walks 16 (head-pair,
q-tile) stages; stage i emits one row-packed score matmul pair (two heads
at PE row offsets 0/64 co-execute) + the ACT exp per k-tile for its own
q-tile, while running the PV matmuls for stage i-1's exp tiles (lag one
stage = 16 iterations, so PV's semaphore waits on the ACT stream are
long-satisfied and PV paces at the PE streaming rate, ~225ns per 512-col
matmul).  The V projection and later head-pairs' Q/K projections drip
into the PE stream between iterations in half-chain granules, sized so
the ACT engine (the hard wall: ~1.07ns/col over the 33.5M-element exp
stream = ~273us busy) starves as little as possible.  PV keeps the
ones-column trick (stationary V_aug [128 kpos, 65] per head; output row
64 is the softmax denominator) -- measured faster than 2-head
column-tiled PV plus separate denominator matmuls, which cannot
co-execute since all four PE column groups are busy during PV.  Inputs
load k-first with q in halves so the first score matmuls wait for ~6MB
of DMA.  PSUM: scores 2x2 banks, PV 2, projections 2 = 8.  Host divides
by the denominator row, adds the V bias (exact: softmax rows sum to 1),
transposes, reassembles.  The V input streams on the scalar DMA queue
concurrently with k/q on sync, and the first 10 of 16 V-projection
chains run inside the prologue's PE-idle window (PE otherwise waits on
the k/q DMA), with the remainder dripped into stages 0-1 -- fully
serializing vproj into the prologue measured worse (72us to first
activation) and fully dripping it starves the ACT engine ~15us in early
stages.  Measured on trn2: ~344us HW exec (v1 baseline 372us), rel err
2.2e-3; PE ~313us busy / ACT ~273us busy.
"""

import numpy as np
import ml_dtypes

import concourse.bacc as bacc
import concourse.tile as tile
from concourse import mybir
from concourse.bass_utils import run_bass_kernel_spmd

F32 = mybir.dt.float32
BF16 = mybir.dt.bfloat16
_BF = ml_dtypes.bfloat16

B, S, D, H, DH = 4, 2048, 1024, 16, 64
HH = 8          # heads per core
NP = HH // 2    # head pairs per core
JW = HH * DH    # 512 projected features per core
N_CORES = 8

KT8 = D // 128   # 8 contraction chunks for projections
NQT = S // 512   # 4 q tiles of 512
NKT = S // 128   # 16 k tiles of 128
NTT = S // 128   # 16 token tiles (v projection)
TC = 512         # projection token chunk
NTC = S // TC
NST = NP * NQT   # 16 (pair, qt) stages


def _build_nc(exp_bufs=22, in_bufs=16):
    nc = bacc.Bacc("TRN2", target_bir_lowering=False, debug=False,
                   num_devices=N_CORES)

    qT = nc.declare_dram_parameter("qT", [D, S], BF16, isOutput=False)
    kT = nc.declare_dram_parameter("kT", [D, S], BF16, isOutput=False)
    vT = nc.declare_dram_parameter("vT", [D, S], BF16, isOutput=False)
    wq = nc.declare_dram_parameter("wq", [D, JW], BF16, isOutput=False)
    wk = nc.declare_dram_parameter("wk", [D, JW], BF16, isOutput=False)
    wv = nc.declare_dram_parameter("wv", [D, JW], BF16, isOutput=False)
    bq = nc.declare_dram_parameter("bq", [JW], F32, isOutput=False)
    bk = nc.declare_dram_parameter("bk", [JW], F32, isOutput=False)
    num = nc.declare_dram_parameter("num", [NP, NQT, 65, 2, 512], F32,
                                    isOutput=True)
    w_dram = {"wq": wq, "wk": wk, "wv": wv}
    in_dram = {"q": qT, "k": kT}

    with tile.TileContext(nc) as tc:
        with (
            tc.tile_pool(name="consts", bufs=1) as consts,
            tc.tile_pool(name="persist", bufs=1) as persist,
            tc.tile_pool(name="ins", bufs=in_bufs) as ins,
            tc.tile_pool(name="vins", bufs=4) as vins,
            tc.tile_pool(name="exps", bufs=exp_bufs) as exps,
            tc.tile_pool(name="ostage", bufs=3) as ostage,
            tc.tile_pool(name="scps", bufs=2, space="PSUM") as scps,
            tc.tile_pool(name="pvps", bufs=2, space="PSUM") as pvps,
            tc.tile_pool(name="prps", bufs=2, space="PSUM") as prps,
        ):
            w_sb = {}

            def load_w(name, eng=None):
                eng = eng or nc.sync
                t = consts.tile([128, KT8, JW], BF16, tag=name)
                src_r = w_dram[name].ap().rearrange("(kt p) j -> p kt j", p=128)
                for h in range(2):
                    eng.dma_start(out=t[:, 4 * h:4 * (h + 1), :],
                                  in_=src_r[:, 4 * h:4 * (h + 1), :])
                w_sb[name] = t

            def load_bias(name, src):
                t = consts.tile([128, NP], F32, tag=name)
                nc.scalar.dma_start(
                    out=t[:], in_=src.ap().rearrange("(pr j) -> j pr", j=128))
                return t

            # Q^T/K^T ring of 2 head-pairs: [128 feat (2 heads), ring, S]
            QT_sb = persist.tile([128, 2, S], BF16, tag="QT")
            KT_sb = persist.tile([128, 2, S], BF16, tag="KT")
            # V_aug: [128 kpos, kt, pair, h2, 65]; [..., 64] = 1.0 (denom)
            V_sb = persist.tile([128, NTT, NP, 2, DH + 1], BF16, tag="V")

            # resident inputs: in_tiles[name] = 8 tiles of [128, S],
            # loaded in two token-halves so early proj chains start sooner
            in_tiles = {}

            def load_input(name, half=None, eng=None):
                if name not in in_tiles:
                    in_tiles[name] = [
                        ins.tile([128, S], BF16, tag="in",
                                 name=f"in_{name}_{kt}")
                        for kt in range(KT8)]
                h0 = 0 if half in (None, 0) else S // 2
                h1 = S if half in (None, 1) else S // 2
                for kt in range(KT8):
                    (eng or nc.sync).dma_start(
                        out=in_tiles[name][kt][:, h0:h1],
                        in_=in_dram[name].ap()[kt * 128:(kt + 1) * 128, h0:h1])

            v_slots = {}

            def load_v_slot(u):
                """Load token tiles 2u and 2u+1 as one 512B-run DMA."""
                t = vins.tile([128, KT8, 256], BF16, tag="vin")
                src = vT.ap().rearrange("(kt p) s -> p kt s", p=128)
                nc.scalar.dma_start(
                    out=t[:], in_=src[:, :, u * 256:(u + 1) * 256])
                v_slots[u] = t

            _chain_state = {}

            def proj_qk_slot(pair, name, s, part=None):
                """One 512-token chunk of the q/k projection (1 PSUM bank).

                part=0/1 emits only the first/second half of the contraction
                chain (for finer interleaving with the score stream)."""
                bias, dst = {"k": (bias_k, KT_sb), "q": (bias_q, QT_sb)}[name]
                key = (pair, name, s)
                if part in (None, 0):
                    _chain_state[key] = prps.tile(
                        [128, TC], F32, tag="pr", name=f"ps_{pair}_{name}_{s}")
                ps = _chain_state[key]
                tc0 = s * TC
                kts = range(KT8) if part is None else (
                    range(KT8 // 2) if part == 0 else range(KT8 // 2, KT8))
                for kt in kts:
                    nc.tensor.matmul(
                        ps[:], w_sb["w" + name][:, kt, pair * 128:(pair + 1) * 128],
                        in_tiles[name][kt][:, tc0:tc0 + TC],
                        start=(kt == 0), stop=(kt == KT8 - 1))
                if part in (None, 1):
                    nc.vector.tensor_scalar_add(
                        dst[:, pair % 2, tc0:tc0 + TC], ps[:],
                        bias[:, pair:pair + 1])
                    del _chain_state[key]

            def proj_v_slot(tt, part=None):
                """One 128-token chunk of the V projection -> V_sb."""
                key = ("v", tt)
                if part in (None, 0):
                    _chain_state[key] = prps.tile([128, JW], F32, tag="pr",
                                                  name=f"psv_{tt}")
                ps = _chain_state[key]
                vt = v_slots[tt // 2]
                c0 = (tt % 2) * 128
                kts = range(KT8) if part is None else (
                    range(KT8 // 2) if part == 0 else range(KT8 // 2, KT8))
                for kt in kts:
                    nc.tensor.matmul(
                        ps[:],
                        vt[:, kt, c0:c0 + 128],
                        w_sb["wv"][:, kt, :],
                        start=(kt == 0), stop=(kt == KT8 - 1))
                if part in (None, 1):
                    nc.vector.tensor_copy(
                        V_sb[:, tt, :, :, 0:DH],
                        ps[:].rearrange("p (pr h2 d) -> p pr h2 d",
                                        pr=NP, h2=2))
                    del _chain_state[key]

            # ---------------- drip scheduling ----------------
            drip = {}

            def add_drip(st, kt, fn):
                drip.setdefault((st, kt), []).append(fn)

            def emit_scores(pair, qt, kt, ets_store):
                k0 = kt * 128
                q0 = qt * 512
                sc = scps.tile([128, 2, 512], F32, tag="sc")
                for h2 in range(2):
                    nc.tensor.matmul(
                        sc[:, h2, :],
                        KT_sb[h2 * 64:(h2 + 1) * 64, pair % 2, k0:k0 + 128],
                        QT_sb[h2 * 64:(h2 + 1) * 64, pair % 2, q0:q0 + 512],
                        start=True, stop=True)
                et = exps.tile([128, 2, 512], BF16, tag="exp")
                nc.scalar.activation(
                    et[:].rearrange("p a b -> p (a b)"),
                    sc[:].rearrange("p a b -> p (a b)"),
                    mybir.ActivationFunctionType.Exp, scale=0.125)
                ets_store[kt] = et

            def emit_pv(pair, kt, ets_store, pv):
                et = ets_store.pop(kt)
                for h2 in range(2):
                    nc.tensor.matmul(
                        pv[h2][:],
                        V_sb[:, kt, pair, h2, :],
                        et[:, h2, :],
                        start=(kt == 0), stop=(kt == NKT - 1))

            def drain(pair, qt, pv):
                ot = ostage.tile([65, 2, 512], F32, tag="ot")
                for h2 in range(2):
                    nc.vector.tensor_copy(ot[:, h2, :], pv[h2][:])
                nc.sync.dma_start(out=num.ap()[pair, qt, :, :, :], in_=ot[:])

            # ---------------- prologue ----------------
            # k/q stream on the sync DMA queue while wv + v stream on the
            # scalar queue; the entire V projection runs in the PE-idle
            # window while k/q arrive, so no vproj work lands in stages 0-1
            load_input("k", half=0)
            load_w("wk")
            load_w("wv", eng=nc.scalar)
            load_input("q", half=0)
            load_w("wq")
            load_input("k", half=1)
            load_input("q", half=1)
            bias_q = load_bias("bq", bq)
            bias_k = load_bias("bk", bk)
            for u in range(NTT // 2):
                load_v_slot(u)
            nc.vector.memset(V_sb[:, :, :, :, DH:DH + 1], 1.0)
            # first 10 V-projection chains fill the PE-idle window while
            # k/q stream in; the rest drip into stages 0-1
            for tt in range(10):
                proj_v_slot(tt)
            proj_qk_slot(0, "k", 0)          # k tiles 0-3
            proj_qk_slot(0, "q", 0)          # q tile 0

            def add_g(g, fn):
                add_drip(g // NKT, g % NKT, fn)

            def add_chain(g, fn2):
                """Split a projection chain across iterations g and g+1."""
                add_g(g, lambda: fn2(0))
                add_g(g + 1, lambda: fn2(1))

            # pair-0 remaining chunks (k-chunk s needed by iter 4s)
            add_chain(0, lambda pt: proj_qk_slot(0, "k", 1, pt))
            add_chain(4, lambda pt: proj_qk_slot(0, "k", 2, pt))
            add_chain(8, lambda pt: proj_qk_slot(0, "k", 3, pt))
            add_chain(12, lambda pt: proj_qk_slot(0, "q", 1, pt))
            add_chain(30, lambda pt: proj_qk_slot(0, "q", 2, pt))
            add_chain(37, lambda pt: proj_qk_slot(0, "q", 3, pt))
            # remaining V-projection chains (slot t needed by g 16+t)
            for i, t in enumerate(range(10, NTT)):
                add_chain(10 + 2 * i, lambda pt, t=t: proj_v_slot(t, pt))
            # later pairs p: scores start at stage 4p (inputs stay resident)
            for p in range(1, NP):
                b0 = (4 * p - 2) * NKT
                b1 = (4 * p - 1) * NKT
                add_chain(b0 + 8, lambda pt, p=p: proj_qk_slot(p, "k", 0, pt))
                add_chain(b0 + 12, lambda pt, p=p: proj_qk_slot(p, "k", 1, pt))
                add_chain(b1 + 2, lambda pt, p=p: proj_qk_slot(p, "k", 2, pt))
                add_chain(b1 + 6, lambda pt, p=p: proj_qk_slot(p, "k", 3, pt))
                add_chain(b1 + 10, lambda pt, p=p: proj_qk_slot(p, "q", 0, pt))
                add_chain(b1 + 13, lambda pt, p=p: proj_qk_slot(p, "q", 1, pt))
                add_chain((4 * p) * NKT + 6,
                          lambda pt, p=p: proj_qk_slot(p, "q", 2, pt))
                add_chain((4 * p + 1) * NKT + 6,
                          lambda pt, p=p: proj_qk_slot(p, "q", 3, pt))

            # ---------------- pipelined stages ----------------
            ets = [dict() for _ in range(NST)]
            prev = None
            for st in range(NST + 1):
                cur = (st // NQT, st % NQT) if st < NST else None
                pv_t = None
                if prev is not None:
                    pv_t = [pvps.tile([65, 512], F32, tag="pv",
                                      name=f"pv_{prev[0]}_{prev[1]}_{h2}")
                            for h2 in range(2)]
                for kt in range(NKT):
                    if cur is not None:
                        emit_scores(cur[0], cur[1], kt, ets[st])
                    if prev is not None:
                        emit_pv(prev[0], kt, ets[st - 1], pv_t)
                    if cur is not None:
                        for fn in drip.get((st, kt), []):
                            fn()
                if prev is not None:
                    drain(prev[0], prev[1], pv_t)
                prev = cur

    nc.compile()
    return nc


_NC_CACHE = {}


def _get_nc():
    if "nc" not in _NC_CACHE:
        _NC_CACHE["nc"] = _build_nc()
    return _NC_CACHE["nc"]


def _make_in_maps(key, value, query, Wq, bq, Wk, bk, Wv):
    in_maps = []
    for c in range(N_CORES):
        b, hh = c // 2, c % 2
        js = slice(hh * JW, (hh + 1) * JW)
        in_maps.append({
            "qT": np.ascontiguousarray(query[b].T).astype(_BF),
            "kT": np.ascontiguousarray(key[b].T).astype(_BF),
            "vT": np.ascontiguousarray(value[b].T).astype(_BF),
            "wq": np.ascontiguousarray(Wq[:, js]).astype(_BF),
            "wk": np.ascontiguousarray(Wk[:, js]).astype(_BF),
            "wv": np.ascontiguousarray(Wv[:, js]).astype(_BF),
            "bq": np.ascontiguousarray(bq[js], dtype=np.float32),
            "bk": np.ascontiguousarray(bk[js], dtype=np.float32),
        })
    return in_maps


def _assemble(results, bv):
    out = np.empty((B, S, H * DH), np.float32)
    for c in range(N_CORES):
        b, hh = c // 2, c % 2
        num = results[c]["num"]      # [NP, NQT, 65, 2, 512]
        for pair in range(NP):
            for qt in range(NQT):
                q0 = qt * 512
                h0 = hh * JW + pair * 2 * DH
                for h2 in range(2):
                    blk = num[pair, qt, :, h2, :]            # [65, 512]
                    out[b, q0:q0 + 512, h0 + h2 * DH:h0 + (h2 + 1) * DH] = \
                        (blk[0:64, :] / blk[64:65, :]).T
    out += bv.astype(np.float32)
    return out


def kernel(key, value, query, Wq, bq, Wk, bk, Wv, bv, **_run_kwargs):
    key = np.asarray(key, np.float32)
    value = np.asarray(value, np.float32)
    query = np.asarray(query, np.float32)
    nc = _get_nc()
    in_maps = _make_in_maps(key, value, query,
                            np.asarray(Wq, np.float32), np.asarray(bq, np.float32),
                            np.asarray(Wk, np.float32), np.asarray(bk, np.float32),
                            np.asarray(Wv, np.float32))
    res = run_bass_kernel_spmd(nc, in_maps, list(range(N_CORES)), **_run_kwargs)
    out = _assemble(res.results, np.asarray(bv, np.float32))
    if _run_kwargs:
        kernel.last_result = res
    return out
